# revision 7
# baseline (speedup 1.0000x reference)
"""Trainium2 Bass kernel for nn_AdaptedLinear (hypernetwork-adapted linear).

Math (per sample b):
  h = emb_id[HN_ids[b]] + emb_layer[layer_id]                 # [256]
  A = (h @ W_A).reshape(R, IN)    t = A @ x_b                 # [16]
  B = (h @ W_B).reshape(OUT, R)
  out_b = weight @ x_b + B @ t + bias                         # never materialize delta

Distribution across 8 NeuronCores (no collectives -- cross-core collectives
measure 60-100us on this fleet due to launch skew, far above their 5us spec):
  - W_B and weight are sharded by output dim (256 cols per core).
  - W_A is needed in full by every core (t couples all ranks r to every
    output shard); it is replicated but stored in fp8 to cut the dominant
    DMA term.  The LoRA path is ~2.5% of the output magnitude, so fp8's
    ~0.8% relative error there costs ~1e-3 end-to-end relative error.
  - Host does layout prep only (embedding gather, transposes, dtype casts,
    sharding); all O(big) FLOPs run on device.

Device pipeline per core:
  Q[b,(r,d)] = sum_i x[b,i] * Wa3[d,r,i]      (128 matmuls, streams W_A)
  t[b,r]     = sum_d Q[b,r,d] * h[b,d]        (fused mul+reduce DVE ops,
                                               split in 2 groups for overlap)
  B_r[b,o]   = sum_d h[b,d] * Wb3[d,o,r]      (32 matmuls, streams W_B shard)
  lora      += B_r * t[:,r]                   (16 scalar_tensor_tensor DVE ops)
  base[b,o]  = sum_i x[b,i] * weight[o,i] + bias[o]   (17 matmuls, bias via
                                                       an appended ones-row)
  out        = base + lora

All small operands are shipped pre-interleaved in their SBUF [128, F]
layout so every DMA moves contiguous per-partition runs (partition-strided
small DMAs cost ~2000 16-byte descriptors and choke the queues).
"""

import sys

sys.path.insert(0, "/opt/trn_rl_repo")

import numpy as np

import concourse.bass as bass
import concourse.bacc as bacc
import concourse.tile as tile
import concourse.mybir as mybir
from concourse.bass_utils import run_bass_kernel_spmd

IN_F, OUT_F, R = 2048, 2048, 16
HDIM = 256
BATCH = 16
N_CORES = 8
OSH = OUT_F // N_CORES  # 256 output cols per core

# dtype config: (Q path = x/W_A), (B path = h/W_B), (base path = x/weight)
DT_WA = mybir.dt.float8e4
DT_WB = mybir.dt.bfloat16
DT_WT = mybir.dt.float32
WA_SCALE = 256.0  # host multiplies W_A by this before the fp8 cast and
#                   divides h by it (keeps fp8 values in the normal range)

IC_Q = IN_F // 128         # 16 i-chunks for the Q matmuls
IC_BASE = 17               # 16 i-chunks + 1 chunk holding the ones/bias row
KPAD = IC_BASE * 128       # 2176 padded contraction rows for the base path
WB_CH = 4                  # wb arrives in 4 chunked DMAs (4 ranks each)
WT_CH = 4                  # wt arrives in 4 chunked DMAs


def _np_dt(dt):
    return np.dtype(mybir.dt.np(dt))


def _build():
    nc = bacc.Bacc("TRN2", target_bir_lowering=False, debug=False,
                   num_devices=N_CORES)
    f32 = mybir.dt.float32

    # per-core DRAM inputs (small ones pre-interleaved to SBUF layout)
    xt8 = nc.dram_tensor("xt8", [128, IC_Q * BATCH], DT_WA, kind="ExternalInput")
    wa = nc.dram_tensor("wa", [IN_F, R * HDIM], DT_WA, kind="ExternalInput")
    h_in = nc.dram_tensor("h", [BATCH, HDIM], f32, kind="ExternalInput")
    ht = nc.dram_tensor("ht", [128, 2 * BATCH], DT_WB, kind="ExternalInput")
    wb = nc.dram_tensor("wb", [R * HDIM, OSH], DT_WB, kind="ExternalInput")
    xt = nc.dram_tensor("xt", [128, IC_BASE * BATCH], DT_WT, kind="ExternalInput")
    wt = nc.dram_tensor("wt", [KPAD, OSH], DT_WT, kind="ExternalInput")
    out = nc.dram_tensor("out", [BATCH, OSH], f32, kind="ExternalOutput")

    with tile.TileContext(nc) as tc:
        with (
            tc.tile_pool(name="small", bufs=1) as small,
            tc.tile_pool(name="wa_pool", bufs=4) as wa_pool,
            tc.tile_pool(name="big", bufs=1) as big,
            tc.tile_pool(name="ps", bufs=8, space="PSUM") as ps,
        ):
            # ---- small resident tiles (contiguous DMAs) ----
            xt8_sb = small.tile([128, IC_Q * BATCH], DT_WA)
            nc.sync.dma_start(xt8_sb[:], xt8[:])
            h_sb = small.tile([BATCH, HDIM], f32)
            nc.sync.dma_start(h_sb[:], h_in[:])
            ht_sb = small.tile([128, 2 * BATCH], DT_WB)
            nc.sync.dma_start(ht_sb[:], ht[:])
            xt_sb = small.tile([128, IC_BASE * BATCH], DT_WT)
            nc.sync.dma_start(xt_sb[:], xt[:])

            # ---- Q phase: stream W_A, accumulate Q[b, (r,d)] in 8 psum banks
            # Split the i-contraction into two accumulation groups so the
            # t-reduction for group A overlaps group B's matmuls.
            GROUP_A = 12  # ic 0..11 in group A, 12..15 in group B
            q_ps = [ps.tile([BATCH, 512], f32, name=f"q{nb}", tag="ps")
                    for nb in range(8)]
            t_parts = [small.tile([BATCH, R], f32, name=f"tp{g}")
                       for g in range(2)]
            tt_scr = small.tile([BATCH, HDIM], f32)
            wa_r = wa[:].rearrange("(c p) m -> c p m", p=128)
            wa_dmas = []

            def t_reduce(g):
                # t_parts[g][b, r] = sum_d Q[b, (r,d)] * h[b, d]
                for r in range(R):
                    nc.vector.scalar_tensor_tensor(
                        out=tt_scr[:],
                        in0=q_ps[r // 2][:, (r % 2) * HDIM:(r % 2 + 1) * HDIM],
                        scalar=1.0, in1=h_sb[:],
                        op0=mybir.AluOpType.mult, op1=mybir.AluOpType.mult,
                        accum_out=t_parts[g][:, r:r + 1])

            for ic in range(IC_Q):
                wa_t = wa_pool.tile([128, R * HDIM], DT_WA, tag="wa")
                wa_dmas.append(nc.sync.dma_start(wa_t[:], wa_r[ic]))
                for nb in range(8):
                    nc.tensor.matmul(
                        q_ps[nb][:],
                        xt8_sb[:, ic * BATCH:(ic + 1) * BATCH],
                        wa_t[:, nb * 512:(nb + 1) * 512],
                        start=(ic in (0, GROUP_A)),
                        stop=(ic in (GROUP_A - 1, IC_Q - 1)),
                    )
                if ic == GROUP_A - 1:
                    t_reduce(0)
            t_reduce(1)
            t_sb = small.tile([BATCH, R], f32)
            nc.vector.tensor_add(t_sb[:], t_parts[0][:], t_parts[1][:])

            # ---- B phase: stream W_B shard, lora += B_r * t[:, r] ----
            # wb lands in 4 chunked DMAs gated behind the wa stream; each
            # chunk's 4 ranks of matmuls + accumulation follow it.
            wb_sb = big.tile([128, 32 * OSH], DT_WB)
            acc_sb = small.tile([BATCH, OSH], f32)
            RPC = R // WB_CH  # ranks per wb chunk
            for cc in range(WB_CH):
                wb_dma = nc.sync.dma_start(
                    wb_sb[:, cc * 8 * OSH:(cc + 1) * 8 * OSH]
                    .rearrange("p (c m) -> p c m", m=OSH),
                    wb[cc * 8 * 128:(cc + 1) * 8 * 128, :]
                    .rearrange("(c p) m -> p c m", p=128))
                tile.add_dep_helper(wb_dma.ins, wa_dmas[min(12 + cc, 15)].ins,
                                    sync=True, reason="wb after wa stream")
                for r in range(cc * RPC, (cc + 1) * RPC):
                    b_ps = ps.tile([BATCH, 512], f32, name=f"b{r}", tag="ps")
                    for dc in range(2):
                        nc.tensor.matmul(
                            b_ps[:, :OSH],
                            ht_sb[:, dc * BATCH:(dc + 1) * BATCH],
                            wb_sb[:, (r * 2 + dc) * OSH:(r * 2 + dc + 1) * OSH],
                            start=(dc == 0), stop=(dc == 1),
                        )
                    if r == 0:
                        nc.vector.tensor_scalar(
                            out=acc_sb[:], in0=b_ps[:, :OSH],
                            scalar1=t_sb[:, r:r + 1], scalar2=None,
                            op0=mybir.AluOpType.mult,
                        )
                    else:
                        nc.vector.scalar_tensor_tensor(
                            out=acc_sb[:], in0=b_ps[:, :OSH],
                            scalar=t_sb[:, r:r + 1], in1=acc_sb[:],
                            op0=mybir.AluOpType.mult, op1=mybir.AluOpType.add,
                        )

            # ---- base phase: out_base = x @ weight_sh.T + bias ----
            wt_sb = big.tile([128, IC_BASE * OSH], DT_WT)
            base_ps = ps.tile([BATCH, 512], f32, name="base", tag="ps")
            wt_bounds = [0, 5, 9, 13, IC_BASE]
            last_wb = None
            for cc in range(WT_CH):
                lo, hi = wt_bounds[cc], wt_bounds[cc + 1]
                wt_dma = nc.sync.dma_start(
                    wt_sb[:, lo * OSH:hi * OSH]
                    .rearrange("p (c m) -> p c m", m=OSH),
                    wt[lo * 128:hi * 128, :]
                    .rearrange("(c p) m -> p c m", p=128))
                tile.add_dep_helper(wt_dma.ins, wb_dma.ins, sync=True,
                                    reason="wt after wb stream")
                for ic in range(lo, hi):
                    nc.tensor.matmul(
                        base_ps[:, :OSH],
                        xt_sb[:, ic * BATCH:(ic + 1) * BATCH],
                        wt_sb[:, ic * OSH:(ic + 1) * OSH],
                        start=(ic == 0), stop=(ic == IC_BASE - 1),
                    )

            # ---- epilogue: out = base + lora ----
            out_sb = small.tile([BATCH, OSH], f32)
            nc.vector.tensor_add(out_sb[:], base_ps[:, :OSH], acc_sb[:])
            nc.sync.dma_start(out[:], out_sb[:])

    nc.compile()
    return nc


_NC_CACHE = None


def _get_nc():
    global _NC_CACHE
    if _NC_CACHE is None:
        _NC_CACHE = _build()
    return _NC_CACHE


def _interleave(a, p=128):
    """[C*p, F] -> [p, C*F]: the SBUF layout used on device."""
    c = a.shape[0] // p
    return np.ascontiguousarray(
        a.reshape(c, p, a.shape[1]).transpose(1, 0, 2).reshape(p, -1))


def _prep(x, HN_ids, layer_id, weight, bias, emb_id, emb_layer, W_A, W_B):
    """Host-side layout prep + sharding. Returns in_maps for 8 cores."""
    f32 = np.float32
    x = np.asarray(x, f32)
    weight = np.asarray(weight, f32)
    bias = np.asarray(bias, f32)
    emb_id = np.asarray(emb_id, f32)
    emb_layer = np.asarray(emb_layer, f32)
    W_A = np.asarray(W_A, f32)
    W_B = np.asarray(W_B, f32)
    ids = np.asarray(HN_ids).astype(np.int64)
    lid = int(np.asarray(layer_id))

    h = emb_id[ids] + emb_layer[lid]                      # [B, HDIM]

    np_wa, np_wb, np_wt = _np_dt(DT_WA), _np_dt(DT_WB), _np_dt(DT_WT)

    xt8 = _interleave(np.ascontiguousarray(x.T)).astype(np_wa)
    # W_A [d, (r,i)] -> [i, (r,d)] so matmuls contract i on partitions
    wa3 = W_A.reshape(HDIM, R, IN_F)
    wa = np.ascontiguousarray(
        (wa3.transpose(2, 1, 0) * WA_SCALE).reshape(IN_F, R * HDIM)
    ).astype(np_wa)
    ht = _interleave(np.ascontiguousarray(h.T)).astype(np_wb)
    # W_B [d, (o,r)] -> [(r,d), o]
    wb3 = W_B.reshape(HDIM, OUT_F, R)
    wb_full = np.ascontiguousarray(wb3.transpose(2, 0, 1))  # [r, d, o]
    xt_aug = np.zeros((KPAD, BATCH), f32)
    xt_aug[:IN_F] = x.T
    xt_aug[IN_F] = 1.0
    xt_il = _interleave(xt_aug).astype(np_wt)
    wt_full = np.zeros((KPAD, OUT_F), f32)
    wt_full[:IN_F] = weight.T
    wt_full[IN_F] = bias

    in_maps = []
    for c in range(N_CORES):
        sl = slice(c * OSH, (c + 1) * OSH)
        in_maps.append({
            "xt8": xt8,
            "wa": wa,
            "h": np.ascontiguousarray(h / WA_SCALE, f32),
            "ht": ht,
            "wb": np.ascontiguousarray(
                wb_full[:, :, sl]).reshape(R * HDIM, OSH).astype(np_wb),
            "xt": xt_il,
            "wt": np.ascontiguousarray(wt_full[:, sl]).astype(np_wt),
        })
    return in_maps


def kernel(**inputs):
    nc = _get_nc()
    in_maps = _prep(**inputs)
    res = run_bass_kernel_spmd(nc, in_maps, core_ids=list(range(N_CORES)))
    return np.concatenate([res.results[c]["out"] for c in range(N_CORES)],
                          axis=1).astype(np.float32)


def run_traced(inputs, n=3):
    """Timing helper for test.py: returns (exec_times_ns, last_results)."""
    nc = _get_nc()
    in_maps = _prep(**inputs)
    times = []
    res = None
    for _ in range(n):
        res = run_bass_kernel_spmd(nc, in_maps, core_ids=list(range(N_CORES)),
                                   trace=True)
        times.append(res.exec_time_ns)
    return times, res


# revision 9
# speedup vs baseline: 1.1257x; 1.1257x over previous
"""Trainium2 Bass kernel for nn_AdaptedLinear (hypernetwork-adapted linear).

Math (per sample b):
  h = emb_id[HN_ids[b]] + emb_layer[layer_id]                 # [256]
  A = (h @ W_A).reshape(R, IN)    t = A @ x_b                 # [16]
  B = (h @ W_B).reshape(OUT, R)
  out_b = weight @ x_b + B @ t + bias                         # never materialize delta

Distribution across 8 NeuronCores (no collectives -- cross-core collectives
measure 60-100us on this fleet due to launch skew, far above their 5us spec):
  - W_B and weight are sharded by output dim (256 cols per core).
  - W_A is needed in full by every core (t couples all ranks r to every
    output shard); it is replicated but stored in fp8 to cut the dominant
    DMA term.  The LoRA path is ~2.5% of the output magnitude, so fp8's
    ~0.8% relative error there costs ~1e-3 end-to-end relative error.
  - Host does layout prep only (embedding gather, transposes, dtype casts,
    sharding); all O(big) FLOPs run on device.

Device pipeline per core:
  Q[b,(r,d)] = sum_i x[b,i] * Wa3[d,r,i]      (128 matmuls, streams W_A)
  t[b,r]     = sum_d Q[b,r,d] * h[b,d]        (fused mul+reduce DVE ops,
                                               split in 2 groups for overlap)
  B_r[b,o]   = sum_d h[b,d] * Wb3[d,o,r]      (32 matmuls, streams W_B shard)
  lora      += B_r * t[:,r]                   (16 scalar_tensor_tensor DVE ops)
  base[b,o]  = sum_i x[b,i] * weight[o,i] + bias[o]   (17 matmuls, bias via
                                                       an appended ones-row)
  out        = base + lora

All small operands are shipped pre-interleaved in their SBUF [128, F]
layout so every DMA moves contiguous per-partition runs (partition-strided
small DMAs cost ~2000 16-byte descriptors and choke the queues).
"""

import sys

sys.path.insert(0, "/opt/trn_rl_repo")

import numpy as np

import concourse.bass as bass
import concourse.bacc as bacc
import concourse.tile as tile
import concourse.mybir as mybir
from concourse.bass_utils import run_bass_kernel_spmd

IN_F, OUT_F, R = 2048, 2048, 16
HDIM = 256
BATCH = 16
N_CORES = 8
OSH = OUT_F // N_CORES  # 256 output cols per core

# dtype config: (Q path = x/W_A), (B path = h/W_B), (base path = x/weight)
DT_WA = mybir.dt.float8e4
DT_WB = mybir.dt.bfloat16
DT_WT = mybir.dt.float32
WA_SCALE = 256.0  # host multiplies W_A by this before the fp8 cast and
#                   divides h by it (keeps fp8 values in the normal range)

IC_Q = IN_F // 128         # 16 i-chunks for the Q matmuls
IC_BASE = 17               # 16 i-chunks + 1 chunk holding the ones/bias row
KPAD = IC_BASE * 128       # 2176 padded contraction rows for the base path
WB_CH = 4                  # wb arrives in 4 chunked DMAs (4 ranks each)
WT_CH = 4                  # wt arrives in 4 chunked DMAs


def _np_dt(dt):
    return np.dtype(mybir.dt.np(dt))


def _build():
    nc = bacc.Bacc("TRN2", target_bir_lowering=False, debug=False,
                   num_devices=N_CORES)
    f32 = mybir.dt.float32

    # per-core DRAM inputs (small ones pre-interleaved to SBUF layout)
    xt8 = nc.dram_tensor("xt8", [128, IC_Q * BATCH], DT_WA, kind="ExternalInput")
    wa = nc.dram_tensor("wa", [IN_F, R * HDIM], DT_WA, kind="ExternalInput")
    h_in = nc.dram_tensor("h", [BATCH, HDIM], f32, kind="ExternalInput")
    ht = nc.dram_tensor("ht", [128, 2 * BATCH], DT_WB, kind="ExternalInput")
    wb = nc.dram_tensor("wb", [R * HDIM, OSH], DT_WB, kind="ExternalInput")
    xt = nc.dram_tensor("xt", [128, IC_BASE * BATCH], DT_WT, kind="ExternalInput")
    wt = nc.dram_tensor("wt", [KPAD, OSH], DT_WT, kind="ExternalInput")
    out = nc.dram_tensor("out", [BATCH, OSH], f32, kind="ExternalOutput")

    with tile.TileContext(nc) as tc:
        with (
            tc.tile_pool(name="small", bufs=1) as small,
            tc.tile_pool(name="wa_pool", bufs=4) as wa_pool,
            tc.tile_pool(name="big", bufs=1) as big,
            tc.tile_pool(name="ps", bufs=8, space="PSUM") as ps,
        ):
            # ---- small resident tiles (contiguous DMAs) ----
            xt8_sb = small.tile([128, IC_Q * BATCH], DT_WA)
            nc.sync.dma_start(xt8_sb[:], xt8[:])
            h_sb = small.tile([BATCH, HDIM], f32)
            nc.sync.dma_start(h_sb[:], h_in[:])
            ht_sb = small.tile([128, 2 * BATCH], DT_WB)
            nc.sync.dma_start(ht_sb[:], ht[:])
            xt_sb = small.tile([128, IC_BASE * BATCH], DT_WT)
            nc.sync.dma_start(xt_sb[:], xt[:])

            # ---- Q phase: stream W_A (8 paired 1MB DMAs, 2 in flight so
            # packets drain in order), DoubleRow fp8 matmuls accumulate
            # Q[b, (r,d)] in 8 psum banks.  The i-contraction is split into
            # two accumulation groups so group A's t-reduction overlaps
            # group B's matmuls.
            NPAIR = IC_Q // 2          # 8 paired i-chunks
            GROUP_A = 6                # pairs 0..5 = group A, 6..7 = group B
            q_ps = [ps.tile([BATCH, 512], f32, name=f"q{nb}", tag="ps")
                    for nb in range(8)]
            t_parts = [small.tile([BATCH, R], f32, name=f"tp{g}")
                       for g in range(2)]
            tt_scr = small.tile([BATCH, HDIM], f32)
            wa_dmas = []

            def t_reduce(g):
                # t_parts[g][b, r] = sum_d Q[b, (r,d)] * h[b, d]
                for r in range(R):
                    nc.vector.scalar_tensor_tensor(
                        out=tt_scr[:],
                        in0=q_ps[r // 2][:, (r % 2) * HDIM:(r % 2 + 1) * HDIM],
                        scalar=1.0, in1=h_sb[:],
                        op0=mybir.AluOpType.mult, op1=mybir.AluOpType.mult,
                        accum_out=t_parts[g][:, r:r + 1])

            for pc in range(NPAIR):
                wa_t = wa_pool.tile([128, 2 * R * HDIM], DT_WA, tag="wa")
                dma = nc.sync.dma_start(
                    wa_t[:].rearrange("p (k m) -> p k m", k=2),
                    wa[pc * 256:(pc + 1) * 256, :]
                    .rearrange("(k p) m -> p k m", p=128))
                if pc >= 2:
                    tile.add_dep_helper(dma.ins, wa_dmas[pc - 2].ins,
                                        sync=True, reason="wa stream order")
                wa_dmas.append(dma)
                for nb in range(8):
                    nc.tensor.matmul(
                        q_ps[nb][:],
                        xt8_sb[:, pc * 2 * BATCH:(pc * 2 + 2) * BATCH]
                        .rearrange("p (k b) -> p k b", k=2),
                        wa_t[:].rearrange("p (k m) -> p k m", k=2)
                        [:, :, nb * 512:(nb + 1) * 512],
                        start=(pc in (0, GROUP_A)),
                        stop=(pc in (GROUP_A - 1, NPAIR - 1)),
                        perf_mode=mybir.MatmulPerfMode.DoubleRow,
                    )
                if pc == GROUP_A - 1:
                    t_reduce(0)
            t_reduce(1)
            t_sb = small.tile([BATCH, R], f32)
            nc.vector.tensor_add(t_sb[:], t_parts[0][:], t_parts[1][:])

            # ---- B phase: stream W_B shard, lora += B_r * t[:, r] ----
            # wb lands in 4 chunked DMAs gated behind the wa stream; each
            # chunk's 4 ranks of matmuls + accumulation follow it.
            wb_sb = big.tile([128, 32 * OSH], DT_WB)
            acc_sb = small.tile([BATCH, OSH], f32)
            RPC = R // WB_CH  # ranks per wb chunk
            for cc in range(WB_CH):
                wb_dma = nc.sync.dma_start(
                    wb_sb[:, cc * 8 * OSH:(cc + 1) * 8 * OSH]
                    .rearrange("p (c m) -> p c m", m=OSH),
                    wb[cc * 8 * 128:(cc + 1) * 8 * 128, :]
                    .rearrange("(c p) m -> p c m", p=128))
                tile.add_dep_helper(wb_dma.ins, wa_dmas[-1].ins,
                                    sync=True, reason="wb after wa stream")
                for r in range(cc * RPC, (cc + 1) * RPC):
                    b_ps = ps.tile([BATCH, 512], f32, name=f"b{r}", tag="ps")
                    for dc in range(2):
                        nc.tensor.matmul(
                            b_ps[:, :OSH],
                            ht_sb[:, dc * BATCH:(dc + 1) * BATCH],
                            wb_sb[:, (r * 2 + dc) * OSH:(r * 2 + dc + 1) * OSH],
                            start=(dc == 0), stop=(dc == 1),
                        )
                    if r == 0:
                        nc.vector.tensor_scalar(
                            out=acc_sb[:], in0=b_ps[:, :OSH],
                            scalar1=t_sb[:, r:r + 1], scalar2=None,
                            op0=mybir.AluOpType.mult,
                        )
                    else:
                        nc.vector.scalar_tensor_tensor(
                            out=acc_sb[:], in0=b_ps[:, :OSH],
                            scalar=t_sb[:, r:r + 1], in1=acc_sb[:],
                            op0=mybir.AluOpType.mult, op1=mybir.AluOpType.add,
                        )

            # ---- base phase: out_base = x @ weight_sh.T + bias ----
            wt_sb = big.tile([128, IC_BASE * OSH], DT_WT)
            base_ps = ps.tile([BATCH, 512], f32, name="base", tag="ps")
            wt_bounds = [0, 5, 9, 13, IC_BASE]
            last_wb = None
            for cc in range(WT_CH):
                lo, hi = wt_bounds[cc], wt_bounds[cc + 1]
                wt_dma = nc.sync.dma_start(
                    wt_sb[:, lo * OSH:hi * OSH]
                    .rearrange("p (c m) -> p c m", m=OSH),
                    wt[lo * 128:hi * 128, :]
                    .rearrange("(c p) m -> p c m", p=128))
                tile.add_dep_helper(wt_dma.ins, wb_dma.ins, sync=True,
                                    reason="wt after wb stream")
                for ic in range(lo, hi):
                    nc.tensor.matmul(
                        base_ps[:, :OSH],
                        xt_sb[:, ic * BATCH:(ic + 1) * BATCH],
                        wt_sb[:, ic * OSH:(ic + 1) * OSH],
                        start=(ic == 0), stop=(ic == IC_BASE - 1),
                    )

            # ---- epilogue: out = base + lora ----
            out_sb = small.tile([BATCH, OSH], f32)
            nc.vector.tensor_add(out_sb[:], base_ps[:, :OSH], acc_sb[:])
            nc.sync.dma_start(out[:], out_sb[:])

    nc.compile()
    return nc


_NC_CACHE = None


def _get_nc():
    global _NC_CACHE
    if _NC_CACHE is None:
        _NC_CACHE = _build()
    return _NC_CACHE


def _interleave(a, p=128):
    """[C*p, F] -> [p, C*F]: the SBUF layout used on device."""
    c = a.shape[0] // p
    return np.ascontiguousarray(
        a.reshape(c, p, a.shape[1]).transpose(1, 0, 2).reshape(p, -1))


def _prep(x, HN_ids, layer_id, weight, bias, emb_id, emb_layer, W_A, W_B):
    """Host-side layout prep + sharding. Returns in_maps for 8 cores."""
    f32 = np.float32
    x = np.asarray(x, f32)
    weight = np.asarray(weight, f32)
    bias = np.asarray(bias, f32)
    emb_id = np.asarray(emb_id, f32)
    emb_layer = np.asarray(emb_layer, f32)
    W_A = np.asarray(W_A, f32)
    W_B = np.asarray(W_B, f32)
    ids = np.asarray(HN_ids).astype(np.int64)
    lid = int(np.asarray(layer_id))

    h = emb_id[ids] + emb_layer[lid]                      # [B, HDIM]

    np_wa, np_wb, np_wt = _np_dt(DT_WA), _np_dt(DT_WB), _np_dt(DT_WT)

    xt8 = _interleave(np.ascontiguousarray(x.T)).astype(np_wa)
    # W_A [d, (r,i)] -> [i, (r,d)] so matmuls contract i on partitions
    wa3 = W_A.reshape(HDIM, R, IN_F)
    wa = np.ascontiguousarray(
        (wa3.transpose(2, 1, 0) * WA_SCALE).reshape(IN_F, R * HDIM)
    ).astype(np_wa)
    ht = _interleave(np.ascontiguousarray(h.T)).astype(np_wb)
    # W_B [d, (o,r)] -> [(r,d), o]
    wb3 = W_B.reshape(HDIM, OUT_F, R)
    wb_full = np.ascontiguousarray(wb3.transpose(2, 0, 1))  # [r, d, o]
    xt_aug = np.zeros((KPAD, BATCH), f32)
    xt_aug[:IN_F] = x.T
    xt_aug[IN_F] = 1.0
    xt_il = _interleave(xt_aug).astype(np_wt)
    wt_full = np.zeros((KPAD, OUT_F), f32)
    wt_full[:IN_F] = weight.T
    wt_full[IN_F] = bias

    in_maps = []
    for c in range(N_CORES):
        sl = slice(c * OSH, (c + 1) * OSH)
        in_maps.append({
            "xt8": xt8,
            "wa": wa,
            "h": np.ascontiguousarray(h / WA_SCALE, f32),
            "ht": ht,
            "wb": np.ascontiguousarray(
                wb_full[:, :, sl]).reshape(R * HDIM, OSH).astype(np_wb),
            "xt": xt_il,
            "wt": np.ascontiguousarray(wt_full[:, sl]).astype(np_wt),
        })
    return in_maps


def kernel(**inputs):
    nc = _get_nc()
    in_maps = _prep(**inputs)
    res = run_bass_kernel_spmd(nc, in_maps, core_ids=list(range(N_CORES)))
    return np.concatenate([res.results[c]["out"] for c in range(N_CORES)],
                          axis=1).astype(np.float32)


def run_traced(inputs, n=3):
    """Timing helper for test.py: returns (exec_times_ns, last_results)."""
    nc = _get_nc()
    in_maps = _prep(**inputs)
    times = []
    res = None
    for _ in range(n):
        res = run_bass_kernel_spmd(nc, in_maps, core_ids=list(range(N_CORES)),
                                   trace=True)
        times.append(res.exec_time_ns)
    return times, res
